# revision 2
# baseline (speedup 1.0000x reference)
"""DenseCapsule routing kernel for 8 Trainium2 NeuronCores (v3).

Problem: x [B=64, I=2048, Din=8], weight [O=64, I=2048, Dout=16, Din=8]
  x_hat = einsum('oidk,bik->boid', w, x); 3 rounds of dynamic routing
  (softmax over O, weighted i-sum, squash, agreement update); out [B, O, Dout].

v3 strategy (I sharded 8-way, 256 i's per core):
  - S1 (uniform-c iteration): dense GEMM from tightly packed (K=32) operands.
  - Phase A: per-i matmuls, row-group x column-group tiling packs 2 i's per
    128-partition psum tile; full-width batched evicts; x_hat -> HBM bf16 with
    8-i batched DMAs alternating between the HWDGE (sync) and SWDGE (gpsimd)
    paths.
  - Iterations 2/3: ONE streaming pass over x_hat per iteration (b-chunked).
    Logits via multiply + contiguous-halves tree adds (2x DVE mode); chunk-
    local softmax; s-partials via stationary-x_hat matmuls (N=8, o-diagonal)
    accumulated across the whole iteration in one 16KB psum block, extracted
    once with a masked multiply + tree.  One AllReduce per iteration; the
    reduced s comes back in (o,d)-major layout and is flipped with 4 PE
    transposes before the squash.
"""

import sys

sys.path.insert(0, "/opt/trn_rl_repo")

import numpy as np
import ml_dtypes

import concourse.bass as bass
import concourse.tile as tile
from concourse import bacc, mybir
from concourse.bass_utils import run_bass_kernel_spmd

F32 = mybir.dt.float32
BF16 = mybir.dt.bfloat16

B, I, DIN, O, DOUT = 64, 2048, 8, 64, 16
NCORES = 8
IL = I // NCORES          # 256 i's per core
G = IL // 4               # 64 groups of 4 i's
OD = O * DOUT             # 1024
EPS = 1e-8
CB = 16                   # b-chunk size for the fused iteration pass
NCB = B // CB             # 4 chunks
NBLK = B * 8              # 512 (b, oct) psum blocks


def _squash(nc, sq_pool, s_gl, v_out):
    """v_out = squash(s_gl) along d; both [64, 1024] f32 SBUF (b,(o d)) layout."""
    sq = sq_pool.tile([B, OD], F32, tag="sq")
    nc.vector.tensor_tensor(sq, s_gl, s_gl, op=mybir.AluOpType.mult)
    n2 = sq_pool.tile([B, O], F32, tag="n2")
    nc.vector.tensor_reduce(
        n2, sq.rearrange("b (o d) -> b o d", d=DOUT),
        axis=mybir.AxisListType.X, op=mybir.AluOpType.add,
    )
    np1 = sq_pool.tile([B, O], F32, tag="np1")
    nc.vector.tensor_scalar_add(np1, n2, 1.0)
    r1 = sq_pool.tile([B, O], F32, tag="r1")
    nc.vector.reciprocal(r1, np1)
    nrm = sq_pool.tile([B, O], F32, tag="nrm")
    nc.scalar.activation(nrm, n2, mybir.ActivationFunctionType.Sqrt)
    nre = sq_pool.tile([B, O], F32, tag="nre")
    nc.vector.tensor_scalar_add(nre, nrm, EPS)
    r2 = sq_pool.tile([B, O], F32, tag="r2")
    nc.vector.reciprocal(r2, nre)
    sc = sq_pool.tile([B, O], F32, tag="sc")
    nc.vector.tensor_tensor(sc, n2, r1, op=mybir.AluOpType.mult)
    sc2 = sq_pool.tile([B, O], F32, tag="sc2")
    nc.vector.tensor_tensor(sc2, sc, r2, op=mybir.AluOpType.mult)
    sc_b = bass.AP(
        tensor=sc2.tensor, offset=sc2.offset,
        ap=[sc2.ap[0], [sc2.ap[1][0], O], [0, DOUT]],
    )
    nc.vector.tensor_tensor(v_out, s_gl, sc_b, op=mybir.AluOpType.mult)


def _squash2(nc, sq_pool, sgt, v_out):
    """squash in the transposed layout: sgt/v_out [128=(b%16*8+oct), 4, 8, 16]
    f32; partition p covers b = 16*c + p//8, o = 8*(p%8) + r."""
    P = 128
    sq = sq_pool.tile([P, 4, 8, DOUT], F32, tag="q_sq")
    nc.vector.tensor_tensor(sq, sgt, sgt, op=mybir.AluOpType.mult)
    n2 = sq_pool.tile([P, 4, 8], F32, tag="q_n2")
    nc.vector.tensor_reduce(
        n2, sq, axis=mybir.AxisListType.X, op=mybir.AluOpType.add,
    )
    np1 = sq_pool.tile([P, 4, 8], F32, tag="q_np1")
    nc.vector.tensor_scalar_add(np1, n2, 1.0)
    r1 = sq_pool.tile([P, 4, 8], F32, tag="q_r1")
    nc.vector.reciprocal(r1, np1)
    nrm = sq_pool.tile([P, 4, 8], F32, tag="q_nrm")
    nc.scalar.activation(nrm, n2, mybir.ActivationFunctionType.Sqrt)
    nre = sq_pool.tile([P, 4, 8], F32, tag="q_nre")
    nc.vector.tensor_scalar_add(nre, nrm, EPS)
    r2 = sq_pool.tile([P, 4, 8], F32, tag="q_r2")
    nc.vector.reciprocal(r2, nre)
    sc = sq_pool.tile([P, 4, 8], F32, tag="q_sc")
    nc.vector.tensor_tensor(sc, n2, r1, op=mybir.AluOpType.mult)
    sc2 = sq_pool.tile([P, 4, 8], F32, tag="q_sc2")
    nc.vector.tensor_tensor(sc2, sc, r2, op=mybir.AluOpType.mult)
    scb = bass.AP(
        tensor=sc2.tensor, offset=sc2.offset,
        ap=[sc2.ap[0], [8, 4], [1, 8], [0, DOUT]],
    )
    nc.vector.tensor_tensor(v_out, sgt, scb, op=mybir.AluOpType.mult)


def _scatter_v_ap(dst_tile):
    """AP writing [128=(bsub,oct), c, r, d] -> DRAM [b, o*16+d] with
    b = 16c + bsub, o = 8*oct + r."""
    return bass.AP(
        tensor=dst_tile.tensor,
        offset=dst_tile.offset,
        ap=[[OD, 16], [128, 8], [16 * OD, 4], [16, 8], [1, DOUT]],
    )


def build():
    nc = bacc.Bacc()
    xt = nc.declare_dram_parameter("xt", [128, G, B], BF16, isOutput=False)
    wp = nc.declare_dram_parameter("wp", [128, G, OD], BF16, isOutput=False)
    xt2 = nc.declare_dram_parameter("xt2", [32, G, B], BF16, isOutput=False)
    wp2 = nc.declare_dram_parameter("wp2", [32, G, OD], BF16, isOutput=False)
    dm2 = nc.declare_dram_parameter("dm2", [128, 8], F32, isOutput=False)
    ident = nc.declare_dram_parameter("ident", [128, 128], F32, isOutput=False)
    out = nc.declare_dram_parameter("out", [B, O, DOUT], F32, isOutput=True)

    groups = [list(range(NCORES))]

    with tile.TileContext(nc) as tc:
        with (
            tc.tile_pool(name="dram", bufs=1, space="DRAM") as dram,
            tc.tile_pool(name="consts", bufs=1) as consts,
            tc.tile_pool(name="persist", bufs=1) as persist,
            tc.tile_pool(name="small", bufs=1) as small,
        ):
            # DRAM scratch
            xh = dram.tile([IL, B, OD], BF16)             # x_hat cache [i,b,od]
            eed = dram.tile([128, 2, B, O], BF16)         # exp(b2) spill
            sp0 = dram.tile([B, OD], F32, name="sp0")
            sr0 = dram.tile([B, OD], F32, addr_space="Shared", name="sr0")
            spq = [dram.tile([128, 512], F32, name=f"spq{t}") for t in range(2)]
            srq = [
                dram.tile([128, 512], F32, addr_space="Shared", name=f"srq{t}")
                for t in range(2)
            ]
            vbd = [dram.tile([B, OD], BF16, name=f"vbd{t}") for t in range(2)]

            DM2 = consts.tile([128, 8], F32)
            nc.sync.dma_start(out=DM2, in_=dm2[:, :])
            IDN = consts.tile([128, 128], F32)
            nc.sync.dma_start(out=IDN, in_=ident[:, :])

            s_gl = persist.tile([B, OD], F32)
            vv = persist.tile([B, OD], F32)

            # ---------------- Phase S1: uniform-c s1 via dense GEMM --------
            with (
                tc.tile_pool(name="s1x", bufs=1) as s1xp,
                tc.tile_pool(name="s1w", bufs=2) as s1wp,
                tc.tile_pool(name="psS1", bufs=1, space="PSUM") as psS1,
            ):
                XT2 = s1xp.tile([32, G, B], BF16)
                nc.sync.dma_start(out=XT2, in_=xt2[:, :, :])
                s1acc = psS1.tile([B, OD], F32)
                for wc in range(4):
                    WC = s1wp.tile([32, 16, OD], BF16, tag="wc2")
                    nc.sync.dma_start(
                        out=WC, in_=wp2[:, 16 * wc : 16 * wc + 16, :]
                    )
                    for gi in range(16):
                        g = 16 * wc + gi
                        for h in range(2):
                            nc.tensor.matmul(
                                s1acc[:, 512 * h : 512 * h + 512],
                                XT2[:, g, :],
                                WC[:, gi, 512 * h : 512 * h + 512],
                                start=(g == 0),
                                stop=(g == G - 1),
                            )
                s_sb = small.tile([B, OD], F32, tag="s_sb")
                nc.vector.tensor_scalar_mul(s_sb, s1acc, 1.0 / O)
                nc.sync.dma_start(out=sp0, in_=s_sb)

            # AllReduce #1 now; overlaps Phase A.
            nc.gpsimd.collective_compute(
                "AllReduce", mybir.AluOpType.add, replica_groups=groups,
                ins=[sp0[:]], outs=[sr0[:]],
            )

            # ---------------- Phase A: x_hat -> HBM (bf16) -----------------
            with (
                tc.tile_pool(name="xtp", bufs=1) as xtp,
                tc.tile_pool(name="wch", bufs=2) as wchp,
                tc.tile_pool(name="psA", bufs=4, space="PSUM") as psA,
                tc.tile_pool(name="stg", bufs=3) as stgp,
            ):
                XT = xtp.tile([128, G, B], BF16)
                nc.sync.dma_start(out=XT, in_=xt[:, :, :])
                st8 = None
                for ic in range(16):
                    wch = wchp.tile([128, 4, OD], BF16, tag="wch")
                    nc.sync.dma_start(out=wch, in_=wp[:, 4 * ic : 4 * ic + 4, :])
                    for i4 in range(4):
                        g = 4 * ic + i4
                        if g % 2 == 0:
                            st8 = stgp.tile([128, 4, OD], BF16, tag="st8")
                        for half in range(2):
                            pt = psA.tile([128, OD], F32, tag="pt")
                            for r2 in range(2):
                                r = 2 * half + r2
                                p0 = 32 * r
                                cbase = 64 * r2
                                for h in range(2):
                                    nc.tensor.matmul(
                                        pt[cbase : cbase + 64,
                                           512 * h : 512 * h + 512],
                                        XT[p0 : p0 + 8, g, :],
                                        wch[p0 : p0 + 8, i4,
                                            512 * h : 512 * h + 512],
                                        start=True,
                                        stop=True,
                                        tile_position=(p0, cbase),
                                    )
                            slot = 2 * (g % 2) + half
                            if slot % 2 == 0:
                                nc.vector.tensor_copy(st8[:, slot, :], pt)
                            else:
                                nc.scalar.copy(st8[:, slot, :], pt)
                        if g % 2 == 1:
                            gg = g // 2
                            for parity in range(2):
                                src = st8[64 * parity : 64 * parity + 64, :, :]
                                dst = bass.AP(
                                    tensor=xh.tensor,
                                    offset=xh.offset
                                    + (8 * gg + parity) * (B * OD),
                                    ap=[[OD, B], [2 * B * OD, 4], [1, OD]],
                                )
                                if parity == 0:
                                    nc.sync.dma_start(out=dst, in_=src)
                                else:
                                    nc.gpsimd.dma_start(out=dst, in_=src)

            nc.sync.dma_start(out=s_gl, in_=sr0[:])
            with tc.tile_pool(name="sq0", bufs=1) as sqp0:
                _squash(nc, sqp0, s_gl, vv)
                vb = sqp0.tile([B, OD], BF16, tag="vb")
                nc.vector.tensor_copy(vb, vv)
                nc.sync.dma_start(out=vbd[0], in_=vb)

            # ---------------- Routing iterations 2 and 3 (fused pass) ------
            with (
                tc.tile_pool(name="ch", bufs=3) as chp,
                tc.tile_pool(name="vrep", bufs=8) as vrp,
                tc.tile_pool(name="tmp", bufs=1) as tmpp,
                tc.tile_pool(name="tr1", bufs=1) as t1p,
                tc.tile_pool(name="tr2", bufs=1) as t2p,
                tc.tile_pool(name="tr3", bufs=1) as t3p,
                tc.tile_pool(name="dbc", bufs=1) as dbcp,
                tc.tile_pool(name="eec", bufs=2) as eecp,
                tc.tile_pool(name="ccp", bufs=2) as ccp,
                tc.tile_pool(name="es", bufs=2) as esp,
                tc.tile_pool(name="mdp", bufs=1) as mdp,
                tc.tile_pool(name="sgp", bufs=1) as sgp,
                tc.tile_pool(name="sq", bufs=1) as sqp,
            ):
                pools = dict(
                    chp=chp, vrp=vrp, tmpp=tmpp, t1p=t1p, t2p=t2p, t3p=t3p,
                    dbcp=dbcp, eecp=eecp, ccp=ccp, esp=esp, mdp=mdp,
                    sgp=sgp, sqp=sqp,
                )
                bufs = dict(
                    xh=xh, eed=eed, spq=spq, srq=srq, vbd=vbd,
                    DM2=DM2, IDN=IDN, out=out,
                )
                for it in (1, 2):
                    _iteration(nc, tc, it, pools, bufs, groups)
    nc.finalize()
    return nc


def _p1_chunk(nc, pools, bufs, it, cb, ti, CH, VRs):
    """Logits + softmax for one (cb, ti) chunk; returns cc coefficients."""
    eecp, ccp, esp = pools["eecp"], pools["ccp"], pools["esp"]
    tmpp, t1p, t2p, t3p = (
        pools["tmpp"], pools["t1p"], pools["t2p"], pools["t3p"]
    )
    dbcp = pools["dbcp"]
    eed = bufs["eed"]
    E2 = None
    if it == 2:
        E2 = eecp.tile([128, CB, O], BF16, tag="E2")
        esl = bass.AP(
            tensor=eed.tensor,
            offset=eed.offset + ti * (B * O) + CB * cb * O,
            ap=[[2 * B * O, 128], [O, CB], [1, O]],
        )
        nc.sync.dma_start(out=E2, in_=esl)
    DBc = dbcp.tile([128, CB, O], BF16, tag="DBc")
    for os_ in range(8):
        TMP = tmpp.tile([128, CB, 128], BF16, tag="TMP")
        nc.vector.tensor_tensor(
            TMP, CH[:, :, 128 * os_ : 128 * os_ + 128], VRs[os_],
            op=mybir.AluOpType.mult,
        )
        tv = TMP.rearrange("p c (o d) -> p c o d", d=DOUT)
        T1 = t1p.tile([128, CB, 8, 8], BF16, tag="T1")
        nc.vector.tensor_tensor(
            T1, tv[:, :, :, 0:8], tv[:, :, :, 8:16], op=mybir.AluOpType.add
        )
        T2 = t2p.tile([128, CB, 8, 4], BF16, tag="T2")
        nc.vector.tensor_tensor(
            T2, T1[:, :, :, 0:4], T1[:, :, :, 4:8], op=mybir.AluOpType.add
        )
        T3 = t3p.tile([128, CB, 8, 2], BF16, tag="T3")
        nc.vector.tensor_tensor(
            T3, T2[:, :, :, 0:2], T2[:, :, :, 2:4], op=mybir.AluOpType.add
        )
        dbs = DBc[:, :, 8 * os_ : 8 * os_ + 8].unsqueeze(3)
        nc.vector.tensor_tensor(
            dbs, T3[:, :, :, 0:1], T3[:, :, :, 1:2], op=mybir.AluOpType.add
        )
    EX = eecp.tile([128, CB, O], BF16, tag="EX")
    nc.scalar.activation(EX, DBc, mybir.ActivationFunctionType.Exp)
    if it == 1:
        esl = bass.AP(
            tensor=eed.tensor,
            offset=eed.offset + ti * (B * O) + CB * cb * O,
            ap=[[2 * B * O, 128], [O, CB], [1, O]],
        )
        nc.sync.dma_start(out=esl, in_=EX)
        EU = EX
    else:
        EU = eecp.tile([128, CB, O], BF16, tag="EU")
        nc.vector.tensor_tensor(EU, EX, E2, op=mybir.AluOpType.mult)
    es = esp.tile([128, CB], F32, tag="es")
    nc.vector.tensor_reduce(
        es, EU, axis=mybir.AxisListType.X, op=mybir.AluOpType.add
    )
    wr = esp.tile([128, CB], F32, tag="wr")
    nc.vector.reciprocal(wr, es)
    cc = ccp.tile([128, CB, O], BF16, tag="cc")
    wrb = bass.AP(
        tensor=wr.tensor, offset=wr.offset, ap=[wr.ap[0], [1, CB], [0, O]]
    )
    nc.vector.tensor_tensor(cc, EU, wrb, op=mybir.AluOpType.mult)
    return cc


def _iteration(nc, tc, it, pools, bufs, groups):
    chp, vrp, mdp, sgp, sqp = (
        pools["chp"], pools["vrp"], pools["mdp"], pools["sgp"], pools["sqp"]
    )
    xh, eed, spq, srq, vbd = (
        bufs["xh"], bufs["eed"], bufs["spq"], bufs["srq"], bufs["vbd"]
    )
    DM2, IDN, out = bufs["DM2"], bufs["IDN"], bufs["out"]

    with tc.tile_pool(name=f"ps2_{it}", bufs=1, space="PSUM") as ps2p:
        psF = ps2p.tile([128, NBLK, 8], F32)
        for cb in range(NCB):
            VRs = []
            for os_ in range(8):
                VR = vrp.tile([128, CB, 128], BF16, tag="VR")
                vsrc = bass.AP(
                    tensor=vbd[it - 1].tensor,
                    offset=vbd[it - 1].offset + CB * cb * OD + 128 * os_,
                    ap=[[0, 128], [OD, CB], [1, 128]],
                )
                nc.gpsimd.dma_start(out=VR, in_=vsrc)
                VRs.append(VR)
            CHs, ccs = [], []
            for ti in range(2):
                CH = chp.tile([128, CB, OD], BF16, tag="CH")
                csrc = bass.AP(
                    tensor=xh.tensor,
                    offset=xh.offset + ti * 128 * (B * OD) + CB * cb * OD,
                    ap=[[B * OD, 128], [OD, CB], [1, OD]],
                )
                nc.sync.dma_start(out=CH, in_=csrc)
                cc = _p1_chunk(nc, pools, bufs, it, cb, ti, CH, VRs)
                CHs.append(CH)
                ccs.append(cc)
            # s-partials: stationary x_hat matmuls, N=8 o-diagonal.
            # The two i-half matmuls of one block run back-to-back because
            # start=True clears has_written for the whole psum bank.
            for b_loc in range(CB):
                for oc in range(8):
                    blk = (CB * cb + b_loc) * 8 + oc
                    for ti in range(2):
                        nc.tensor.matmul(
                            psF[:, blk, :],
                            CHs[ti][:, b_loc, 128 * oc : 128 * oc + 128],
                            ccs[ti][:, b_loc, 8 * oc : 8 * oc + 8],
                            start=(ti == 0),
                            stop=(ti == 1),
                        )
        # extract s-partials (masked multiply + tree over the 8 o' columns)
        md = mdp.tile([128, NBLK, 8], F32, tag="md")
        dmb = bass.AP(
            tensor=DM2.tensor, offset=DM2.offset,
            ap=[DM2.ap[0], [0, NBLK], [1, 8]],
        )
        nc.vector.tensor_tensor(md, psF, dmb, op=mybir.AluOpType.mult)
        m1 = mdp.tile([128, NBLK, 4], F32, tag="m1")
        nc.vector.tensor_tensor(
            m1, md[:, :, 0:4], md[:, :, 4:8], op=mybir.AluOpType.add
        )
        m2 = mdp.tile([128, NBLK, 2], F32, tag="m2")
        nc.vector.tensor_tensor(
            m2, m1[:, :, 0:2], m1[:, :, 2:4], op=mybir.AluOpType.add
        )
        sPart = mdp.tile([128, NBLK], F32, tag="sP")
        nc.vector.tensor_tensor(
            sPart.unsqueeze(2), m2[:, :, 0:1], m2[:, :, 1:2],
            op=mybir.AluOpType.add,
        )
        nc.sync.dma_start(out=spq[it - 1], in_=sPart)

    nc.gpsimd.collective_compute(
        "AllReduce", mybir.AluOpType.add, replica_groups=groups,
        ins=[spq[it - 1][:]], outs=[srq[it - 1][:]],
    )
    SG = sgp.tile([128, 512], F32, tag="SG")
    nc.sync.dma_start(out=SG, in_=srq[it - 1][:])
    with tc.tile_pool(name=f"psT_{it}", bufs=1, space="PSUM") as psTp:
        PST = psTp.tile([128, 4, 128], F32)
        for c in range(4):
            nc.tensor.transpose(
                PST[:, c, :], SG[:, 128 * c : 128 * c + 128], IDN
            )
        SGT = sgp.tile([128, 4, 8, DOUT], F32, tag="SGT")
        nc.vector.tensor_copy(SGT.rearrange("p c r d -> p c (r d)"), PST)
    VV2 = sqp.tile([128, 4, 8, DOUT], F32, tag="VV2")
    _squash2(nc, sqp, SGT, VV2)
    if it == 1:
        VB = sqp.tile([128, 4, 8, DOUT], BF16, tag="VB")
        nc.vector.tensor_copy(VB, VV2)
        nc.sync.dma_start(out=_scatter_v_ap(vbd[1]), in_=VB)
    else:
        oh = out[:, :, :]
        oap = bass.AP(
            tensor=oh.tensor, offset=oh.offset,
            ap=[[OD, 16], [128, 8], [16 * OD, 4], [16, 8], [1, DOUT]],
        )
        nc.sync.dma_start(out=oap, in_=VV2)


def _pack_inputs(x, weight):
    """Host-side packing of per-core shards (numpy, bf16)."""
    bf = ml_dtypes.bfloat16
    xv = x.reshape(B, NCORES, G, 4, DIN)             # b, c, g, r, k
    xt = np.zeros((NCORES, 4, 32, G, B), np.float32)
    xt[:, :, :DIN] = xv.transpose(1, 3, 4, 2, 0)     # c, r, k, g, b
    xt = xt.reshape(NCORES, 128, G, B).astype(bf)
    xt2 = (
        xv.transpose(1, 3, 4, 2, 0).reshape(NCORES, 32, G, B).astype(bf)
    )
    wv = weight.reshape(O, NCORES, G, 4, DOUT, DIN)  # o, c, g, r, d, k
    wp = np.zeros((NCORES, 4, 32, G, O, DOUT), np.float32)
    wp[:, :, :DIN] = wv.transpose(1, 3, 5, 2, 0, 4)  # c, r, k, g, o, d
    wp = wp.reshape(NCORES, 128, G, OD).astype(bf)
    wp2 = (
        wv.transpose(1, 3, 5, 2, 0, 4).reshape(NCORES, 32, G, OD).astype(bf)
    )
    dm_ = np.zeros((128, 8), np.float32)
    for p in range(128):
        dm_[p, p // 16] = 1.0
    idn = np.eye(128, dtype=np.float32)
    return xt, wp, xt2, wp2, dm_, idn


_CACHE = {}


def _make_runner(nc, key, nruns=1):
    ck = (key, nruns)
    if ck in _CACHE:
        return _CACHE[ck]
    import jax
    from jax.sharding import Mesh, PartitionSpec, NamedSharding
    from jax.experimental.shard_map import shard_map
    from concourse import bass2jax as b2j

    b2j.install_neuronx_cc_hook()
    partition_name = nc.partition_id_tensor.name if nc.partition_id_tensor else None
    in_names, out_names, out_avals, zero_outs = [], [], [], []
    for alloc in nc.m.functions[0].allocations:
        if not isinstance(alloc, mybir.MemoryLocationSet):
            continue
        name = alloc.memorylocations[0].name
        if alloc.kind == "ExternalInput":
            if name != partition_name:
                in_names.append(name)
        elif alloc.kind == "ExternalOutput":
            out_names.append(name)
            shape = tuple(alloc.tensor_shape)
            dtype = mybir.dt.np(alloc.dtype)
            out_avals.append(jax.core.ShapedArray(shape, dtype))
            zero_outs.append(np.zeros(shape, dtype))
    assert len(out_names) == 1
    n_params = len(in_names)
    all_names = list(in_names) + list(out_names)
    if partition_name is not None:
        all_names.append(partition_name)
    donate = (n_params,)

    def _body(*args):
        params = list(args[:n_params])
        z = args[n_params]
        for _ in range(nruns):
            operands = params + [z]
            if partition_name is not None:
                operands.append(b2j.partition_id_tensor())
            (z,) = b2j._bass_exec_p.bind(
                *operands,
                out_avals=tuple(out_avals),
                in_names=tuple(all_names),
                out_names=tuple(out_names),
                lowering_input_output_aliases=(),
                sim_require_finite=True,
                sim_require_nnan=True,
                nc=nc,
            )
        return (z,)

    devices = jax.devices()[:NCORES]
    mesh = Mesh(np.asarray(devices), ("core",))
    in_specs = (PartitionSpec("core"),) * (n_params + 1)
    out_specs = (PartitionSpec("core"),)
    sharded = jax.jit(
        shard_map(_body, mesh=mesh, in_specs=in_specs, out_specs=out_specs,
                  check_rep=False),
        donate_argnums=donate, keep_unused=True,
    )
    sharding = NamedSharding(mesh, PartitionSpec("core"))

    def put_inputs(in_maps):
        return [
            jax.device_put(
                np.concatenate(
                    [np.asarray(in_maps[c][nm]) for c in range(NCORES)], axis=0
                ),
                sharding,
            )
            for nm in in_names
        ]

    def run(dev_in):
        z = np.zeros(
            (NCORES * zero_outs[0].shape[0], *zero_outs[0].shape[1:]),
            zero_outs[0].dtype,
        )
        (o,) = sharded(*dev_in, z)
        o = jax.block_until_ready(o)
        return np.asarray(o).reshape(NCORES, *out_avals[0].shape)

    r = (put_inputs, run)
    _CACHE[ck] = r
    _CACHE[f"sharded_{key}"] = sharded
    _CACHE[f"zshape_{key}"] = (
        NCORES * zero_outs[0].shape[0], *zero_outs[0].shape[1:]
    )
    return r


def _in_maps(x, weight):
    xt, wp, xt2, wp2, dm_, idn = _pack_inputs(
        np.asarray(x, dtype=np.float32), np.asarray(weight, dtype=np.float32)
    )
    return [
        {"xt": xt[c], "wp": wp[c], "xt2": xt2[c], "wp2": wp2[c],
         "dm2": dm_, "ident": idn}
        for c in range(NCORES)
    ]


def kernel(x, weight):
    if "nc" not in _CACHE:
        _CACHE["nc"] = build()
    put, run = _make_runner(_CACHE["nc"], "main", 1)
    outs = run(put(_in_maps(x, weight)))
    return np.asarray(outs[0], dtype=np.float32)


def measure(x, weight, nqueue=32, reps=3):
    import time
    import jax
    if "nc" not in _CACHE:
        _CACHE["nc"] = build()
    nc = _CACHE["nc"]
    maps = _in_maps(x, weight)
    put1, run1 = _make_runner(nc, "main", 1)
    dev = put1(maps)
    sharded = _CACHE["sharded_main"]
    zshape = _CACHE["zshape_main"]

    def chain(k):
        z = np.zeros(zshape, np.float32)
        for _ in range(k):
            (z,) = sharded(*dev, z)
        return z

    jax.block_until_ready(chain(2))
    t1s, tks = [], []
    for _ in range(reps):
        t0 = time.perf_counter_ns()
        jax.block_until_ready(chain(1))
        t1s.append(time.perf_counter_ns() - t0)
        t0 = time.perf_counter_ns()
        jax.block_until_ready(chain(nqueue))
        tks.append(time.perf_counter_ns() - t0)
    per_exec = (min(tks) - min(t1s)) / (nqueue - 1)
    return int(per_exec), min(t1s), min(tks)
